# revision 13
# baseline (speedup 1.0000x reference)
"""ChannelAttention TRN2 Bass kernel.

Math (per sample):
  xf = x.reshape(C, L)
  G  = xf @ xf.T                      [C, C]   (Gram over spatial dim)
  S  = Wq @ G @ Wk.T                  [C, C]   (== Q @ K.T)
  A  = softmax(S, axis=1)
  M  = A @ Wv                         [C, C]
  y  = gama * (M @ xf) + xf           [C, L]

Sharding: data-parallel over batch (16 samples / 8 cores = 2 per core).

Precision: Gram + value path in fp16 (PE runs fp16 at 1 cyc/row vs 4 for
fp32); S-chain (small [512,512] matmuls), softmax, and the +x residual in
fp32.

Schedule: phases of the two samples are interleaved to hide softmax /
copy latencies behind PE matmul work:
  TG0 S0 TG1 AT0 MT0 S1 OUT0 AT1 MT1 OUT1
where TG streams x column-chunks, transposes them on the PE and
immediately accumulates the Gram matrix per chunk.
"""
import numpy as np

N_CORES = 8
N, C, H, W = 16, 512, 64, 64
L = H * W            # 4096
NS = N // N_CORES    # samples per core
P = 128              # partitions
KT = C // P          # 4 c-tiles
LT = L // P          # 32 l-tiles
NCH = L // 512       # 8 column chunks of 512


def _build(trace_scopes=False, repeat=None):
    import concourse.bass as bass
    import concourse.mybir as mybir
    import concourse.tile as tile
    from concourse import bacc
    from concourse.masks import make_identity
    from concourse.bass import ds

    f32 = mybir.dt.float32
    f16 = mybir.dt.float16
    AF = mybir.ActivationFunctionType

    nc = bacc.Bacc("TRN2", debug=False)
    x_d = nc.dram_tensor("x", [NS, C, L], f32, kind="ExternalInput")
    wq_d = nc.dram_tensor("Wq", [C, C], f32, kind="ExternalInput")
    wk_d = nc.dram_tensor("Wk", [C, C], f32, kind="ExternalInput")
    wv_d = nc.dram_tensor("Wv", [C, C], f32, kind="ExternalInput")
    gama_d = nc.dram_tensor("gama", [1], f32, kind="ExternalInput")
    y_d = nc.dram_tensor("y", [NS, C, L], f32, kind="ExternalOutput")

    with tile.TileContext(nc) as tc:
        from contextlib import ExitStack
        ctx = ExitStack()
        with ctx:
            consts = ctx.enter_context(tc.tile_pool(name="consts", bufs=1))
            wpool = ctx.enter_context(tc.tile_pool(name="wpool", bufs=1))
            ypool = ctx.enter_context(tc.tile_pool(name="ypool", bufs=2))
            sbuf = ctx.enter_context(tc.tile_pool(name="sbuf", bufs=1))
            gpool = ctx.enter_context(tc.tile_pool(name="gpool", bufs=2))
            stream = ctx.enter_context(tc.tile_pool(name="stream", bufs=3))
            stage = ctx.enter_context(tc.tile_pool(name="stage", bufs=2))
            stats = ctx.enter_context(tc.tile_pool(name="stats", bufs=4))
            ps = ctx.enter_context(tc.tile_pool(name="ps", bufs=1, space="PSUM"))

            ident = consts.tile([P, P], f32)
            make_identity(nc, ident[:])
            ident16 = consts.tile([P, P], f16)
            make_identity(nc, ident16[:])
            gama_sb = consts.tile([P, 1], f32)
            nc.gpsimd.dma_start(out=gama_sb[:], in_=gama_d.ap().to_broadcast((P, 1)))

            # --- weights ---
            wv16_sb = wpool.tile([P, KT, C], f16)  # Wv natural: [k part, k-tile, c]
            wvn = stream.tile([P, KT, C], f32, tag="stream", name="wvn", bufs=3)
            nc.sync.dma_start(
                out=wvn[:], in_=wv_d.ap().rearrange("(t p) c -> p t c", p=P))
            nc.vector.tensor_copy(out=wv16_sb[:], in_=wvn[:])
            wqT_sb = wpool.tile([P, KT, C], f32)   # Wq^T: [c part, c-tile, q]
            wkT_sb = wpool.tile([P, KT, C], f32)
            for w_d, wT_sb in ((wq_d, wqT_sb), (wk_d, wkT_sb)):
                wn = stream.tile([P, KT, C], f32, tag="stream", name="wn", bufs=3)
                nc.sync.dma_start(out=wn[:], in_=w_d.ap().rearrange("(t p) c -> p t c", p=P))
                for ct in range(KT):
                    ptw = ps.tile([P, C], f32, tag="pt", bufs=2, name="ptw")
                    for qt in range(KT):
                        nc.tensor.transpose(
                            ptw[:, ds(qt * P, P)], wn[:, qt, ds(ct * P, P)], ident[:])
                    nc.scalar.copy(out=wT_sb[:, ct, :], in_=ptw[:])

            # per-sample state kept across interleaved phases
            gs = [None] * NS
            hts = [None] * NS
            es = [None] * NS
            a16s = [None] * NS
            ats = [None] * NS
            mts = [None] * NS

            def x_re(s):
                return x_d.ap()[s].rearrange("(t p) l -> p t l", p=P)

            def phase_TG(s):
                """Stream x, transpose on PE (fp16), accumulate G per chunk."""
                x_s = x_re(s)
                y_sb = ypool.tile([P, LT, C], f16, tag="Y", name=f"y{s}")
                g_ps = [ps.tile([P, C], f32, tag="acc", bufs=4, name=f"g{s}_{m}")
                        for m in range(KT)]
                for lc in range(NCH):
                    xs32 = stream.tile([P, KT, 512], f32, tag="stream",
                                       name="xs32", bufs=3)
                    nc.sync.dma_start(out=xs32[:], in_=x_s[:, :, ds(lc * 512, 512)])
                    xs = stream.tile([P, KT, 512], f16, tag="stream16",
                                     name="xs", bufs=3)
                    nc.vector.tensor_copy(out=xs[:], in_=xs32[:])
                    for j in range(4):
                        lt = lc * 4 + j
                        pt = ps.tile([P, C], f16, tag="pt", bufs=2, name="pt")
                        for ci in range(KT):
                            nc.tensor.transpose(
                                pt[:, ds(ci * P, P)], xs[:, ci, ds(j * P, P)],
                                ident16[:])
                        nc.scalar.copy(out=y_sb[:, lt, :], in_=pt[:])
                    for j in range(4):
                        lt = lc * 4 + j
                        for m in range(KT):
                            nc.tensor.matmul(
                                g_ps[m][:], y_sb[:, lt, ds(m * P, P)],
                                y_sb[:, lt, :],
                                start=(lt == 0), stop=(lt == LT - 1))
                g_sb = gpool.tile([P, KT, C], f32, tag="G", name=f"gsb{s}")
                for m in range(KT):
                    nc.vector.tensor_copy(out=g_sb[:, m, :], in_=g_ps[m][:])
                gs[s] = g_sb

            def phase_S(s):
                """S = Wq G Wk^T and softmax -> A (fp16)."""
                g_sb = gs[s]
                ht_sb = sbuf.tile([P, KT, C], f32, tag="HT", name=f"ht{s}")
                e_sb = sbuf.tile([P, KT, C], f32, tag="E", name=f"e{s}")
                a16_sb = sbuf.tile([P, KT, C], f16, tag="A", name=f"a{s}")
                for m in range(KT):
                    ht_ps = ps.tile([P, C], f32, tag="mm", bufs=2, name="ht_ps")
                    for k in range(KT):
                        nc.tensor.matmul(
                            ht_ps[:], g_sb[:, k, ds(m * P, P)], wqT_sb[:, k, :],
                            start=(k == 0), stop=(k == KT - 1))
                    nc.scalar.copy(out=ht_sb[:, m, :], in_=ht_ps[:])
                for m in range(KT):
                    s_ps = ps.tile([P, C], f32, tag="mm", bufs=2, name="s_ps")
                    for k in range(KT):
                        nc.tensor.matmul(
                            s_ps[:], ht_sb[:, k, ds(m * P, P)], wkT_sb[:, k, :],
                            start=(k == 0), stop=(k == KT - 1))
                    negmax = stats.tile([P, 1], f32, tag="negmax", name="negmax")
                    nc.vector.reduce_max(
                        out=negmax[:], in_=s_ps[:], axis=mybir.AxisListType.X,
                        negate=True)
                    rowsum = stats.tile([P, 1], f32, tag="rowsum", name="rowsum")
                    nc.scalar.activation(
                        out=e_sb[:, m, :], in_=s_ps[:], func=AF.Exp,
                        bias=negmax[:], scale=1.0, accum_out=rowsum[:])
                    rinv = stats.tile([P, 1], f32, tag="rinv", name="rinv")
                    nc.vector.reciprocal(out=rinv[:], in_=rowsum[:])
                    nc.vector.tensor_scalar_mul(
                        a16_sb[:, m, :], e_sb[:, m, :], rinv[:])
                hts[s] = ht_sb
                es[s] = e_sb
                a16s[s] = a16_sb

            def phase_AT(s):
                a16_sb = a16s[s]
                at_sb = sbuf.tile([P, KT, C], f16, tag="AT", name=f"at{s}")
                for kt in range(KT):
                    at_ps = ps.tile([P, C], f16, tag="pt", bufs=2, name="at_ps")
                    for qi in range(KT):
                        nc.tensor.transpose(
                            at_ps[:, ds(qi * P, P)], a16_sb[:, qi, ds(kt * P, P)],
                            ident16[:])
                    nc.scalar.copy(out=at_sb[:, kt, :], in_=at_ps[:])
                ats[s] = at_sb

            def phase_MT(s):
                at_sb = ats[s]
                mt_sb = sbuf.tile([P, KT, C], f16, tag="MT", name=f"mt{s}")
                for m in range(KT):
                    mt_ps = ps.tile([P, C], f32, tag="mm", bufs=2, name="mt_ps")
                    for k in range(KT):
                        nc.tensor.matmul(
                            mt_ps[:], wv16_sb[:, k, ds(m * P, P)], at_sb[:, k, :],
                            start=(k == 0), stop=(k == KT - 1))
                    nc.scalar.activation(
                        out=mt_sb[:, m, :], in_=mt_ps[:], func=AF.Copy,
                        bias=0.0, scale=gama_sb[:])
                mts[s] = mt_sb

            def phase_OUT(s):
                x_s = x_re(s)
                y_s = y_d.ap()[s].rearrange("(t p) l -> p t l", p=P)
                mt_sb = mts[s]
                for ncx in range(NCH):
                    xn = stream.tile([P, KT, 512], f32, tag="stream", name="xn")
                    nc.sync.dma_start(out=xn[:], in_=x_s[:, :, ds(ncx * 512, 512)])
                    xn16 = stream.tile([P, KT, 512], f16, tag="stream16",
                                       name="xn16")
                    nc.scalar.copy(out=xn16[:], in_=xn[:])
                    o_ps = [ps.tile([P, 512], f32, tag="acc", bufs=4,
                                    name=f"o{m}") for m in range(KT)]
                    for m in range(KT):
                        for k in range(KT):
                            nc.tensor.matmul(
                                o_ps[m][:], mt_sb[:, k, ds(m * P, P)],
                                xn16[:, k, :],
                                start=(k == 0), stop=(k == KT - 1))
                    stg = stage.tile([P, KT, 512], f32, tag="stage", name="stg")
                    for m in range(KT):
                        nc.vector.tensor_add(stg[:, m, :], o_ps[m][:],
                                             xn[:, m, :])
                    nc.sync.dma_start(out=y_s[:, :, ds(ncx * 512, 512)],
                                      in_=stg[:])

            # interleaved schedule over the two samples
            import os
            if repeat is None:
                repeat = int(os.environ.get("KERNEL_BUILD_REPEAT", "1"))
            for _rep in range(repeat):
                phase_TG(0)
                phase_S(0)
                phase_TG(1)
                phase_AT(0)
                phase_MT(0)
                phase_S(1)
                phase_OUT(0)
                phase_AT(1)
                phase_MT(1)
                phase_OUT(1)

    nc.finalize()
    return nc


_NC_CACHE = {}


def _get_nc():
    if "nc" not in _NC_CACHE:
        _NC_CACHE["nc"] = _build()
    return _NC_CACHE["nc"]


def _run(inputs, trace=False):
    from concourse.bass_utils import run_bass_kernel_spmd

    x = np.ascontiguousarray(np.asarray(inputs["x"], dtype=np.float32)
                             .reshape(N, C, L))
    wq = np.ascontiguousarray(np.asarray(inputs["Wq"], dtype=np.float32))
    wk = np.ascontiguousarray(np.asarray(inputs["Wk"], dtype=np.float32))
    wv = np.ascontiguousarray(np.asarray(inputs["Wv"], dtype=np.float32))
    gama = np.ascontiguousarray(np.asarray(inputs["gama"], dtype=np.float32)
                                .reshape(1))

    nc = _get_nc()
    in_maps = [
        {"x": x[c * NS:(c + 1) * NS], "Wq": wq, "Wk": wk, "Wv": wv, "gama": gama}
        for c in range(N_CORES)
    ]
    res = run_bass_kernel_spmd(nc, in_maps, core_ids=list(range(N_CORES)),
                               trace=trace)
    y = np.concatenate([r["y"][None] for r in res.results], axis=0)
    y = y.reshape(N, C, H, W).astype(np.float32)
    return y, res


def kernel(**inputs):
    y, _ = _run(inputs, trace=False)
    return y
